# revision 37
# baseline (speedup 1.0000x reference)
"""AGCA (adaptive graph channel attention) distributed Bass kernel for TRN2.

Reference computation (per batch row b):
    y   = mean(x[b], axis=(H,W))                    # [CIN]
    y1  = W1 @ y                                    # [HIDE]
    A1  = softmax(w2 * y1)                          # [HIDE]
    y2  = y1 * A1 + A2.T-contract(y1)               # y1@A2
    y3  = relu(w3 * y2)
    out = sigmoid(W4 @ y3)                          # [OP]

Sharding: pure data-parallel over batch. Each of the 8 cores handles
B/8 = 8 batch rows end-to-end; the tiny params are replicated. No
collectives. The kernel is memory-bound on streaming x (64 MiB/core).

Per-core dataflow:
  - x shard viewed as [BL=8, CT=4, 128, 4096] (batch, channel-tile,
    channel-within-tile, H*W), streamed ct-major as ~4 MiB supertiles
    through an NBUF-deep SBUF rotation via SWDGE DMA that casts
    f32 -> bf16 in the datapath (halves SBUF-AXI write traffic; the
    stream runs at the per-core HBM read limit, ~425 GB/s).
  - The vector engine sum-reduces each supertile along the free axis
    into yT tiles [128c, 8b] (f32 accumulation; the 1/4096 mean scale
    is folded into W1 on the host). Every 3rd body reduce and the two
    slowest tail reduces ride the otherwise-idle scalar engine
    (activation Copy with accum_out), so both engines keep >30% margin
    over the DMA cadence even when engine clocks are throttled ~20% by
    co-tenant load. The final batch row is tapered geometrically along
    hw, so the post-last-byte reduce work is under 1 us.
  - Per channel tile, its W1 matmuls run mid-stream on the tensor
    engine (y1 [8,128] and y1T [128,8] layouts both computed so softmax
    runs along the free axis). The epilogue reads y1 straight from
    PSUM: exp with fused accum (softmax denominator) on ACT, the
    normalize/A2/relu chain on DVE+PE, and sigmoid as
    0.5*tanh(z/2)+0.5 (tanh shares the exp LUT set, so no table load
    sits on the critical path).
  - Output [8, 512] (batch-major) DMAd out; host concatenates shards.
"""

import numpy as np

import concourse.bass as bass
import concourse.mybir as mybir
from concourse.bass_utils import run_bass_kernel_spmd


def _install_ntff_shim():
    """Fill in the optional antenv.axon_hooks module if the image lacks it,
    so run_bass_kernel_spmd(trace=True) (or BASS_TRACE=1) can drive NTFF
    profiling through libaxon_pjrt.so instead of crashing on the import.
    No-op when the module exists or the axon .so is unavailable."""
    import sys as _sys
    import types as _types

    if "antenv.axon_hooks" in _sys.modules:
        return
    try:
        import antenv  # noqa: F401
        import importlib.util as _ilu

        if _ilu.find_spec("antenv.axon_hooks") is not None:
            return
        mod = _types.ModuleType("antenv.axon_hooks")
        _hook = [None]
        mod.set_axon_ntff_profile_hook = lambda h: _hook.__setitem__(0, h)
        mod.get_axon_ntff_profile_hook = lambda: _hook[0]
        try:
            from trn_agent_boot.trn_boot import _ntff_profile_via_ctypes

            mod.set_axon_ntff_profile_hook(
                _ntff_profile_via_ctypes("/opt/axon/libaxon_pjrt.so")
            )
        except Exception:
            pass  # hook stays None; bass_utils logs and skips tracing
        _sys.modules["antenv.axon_hooks"] = mod
        antenv.axon_hooks = mod
    except Exception:
        pass


_install_ntff_shim()

F32 = mybir.dt.float32

B, CIN, H, W = 64, 512, 64, 64
HW = H * W          # 4096
NCORES = 8
BL = B // NCORES    # 8 batch rows per core
CT = CIN // 128     # 4 channel tiles
HIDE = 128
OP = 512
NBST = 2            # batch rows per (full) supertile
NBUF = 5            # streaming buffers


NTAPER = 5  # taper chunks for the very last batch row (1 ACT + 4 DVE)


def make_jobs(hw, nbuf):
    """Streaming schedule.

    Each job dict: b0, nb, ct, hw0, nhw, eng ('V' DVE / 'A' ACT reduce),
    dst ('yt', b0, nb) | ('ytx', k), slot (buffer index), boff (element
    offset within the slot's hw axis), q ('S' sync / 'C' scalar HWDGE
    queue), sem (completion-sem index), wait (completion threshold),
    gate (job index whose consumer must finish before this DMA issues).

    Body jobs rotate through the nbuf slots and alternate DMA queues.
    Tail jobs (b=BL-1 taper + the b=BL-2 single) use slot 0 / slot 1
    sub-regions with private sems so nothing gates on late reduces.
    """
    raw = []

    def add(**kw):
        raw.append(dict(kw))

    bi = 0  # body index; every 3rd body reduce rides ACT so the DVE
    # keeps a wide margin even when engine clocks are throttled ~20%.

    def beng():
        nonlocal bi
        e = 'A' if bi % 3 == 0 else 'V'
        bi += 1
        return e

    for ct in range(CT):
        if ct < CT - 1:
            for b0 in range(0, BL, NBST):
                add(b0=b0, nb=NBST, ct=ct, hw0=0, nhw=hw, eng=beng(),
                    dst=('yt', b0, NBST))
        else:
            for b0 in range(0, BL - NBST, NBST):
                add(b0=b0, nb=NBST, ct=ct, hw0=0, nhw=hw, eng=beng(),
                    dst=('yt', b0, NBST))
            add(b0=BL - 2, nb=1, ct=ct, hw0=0, nhw=hw, eng='A',
                dst=('yt', BL - 2, 1))
            ch = [hw // 2, hw // 4, hw // 8, hw // 16, hw // 16]
            assert len(ch) == NTAPER and sum(ch) == hw
            hw0 = 0
            for k in range(NTAPER):
                add(b0=BL - 1, nb=1, ct=ct, hw0=hw0, nhw=ch[k],
                    eng='A' if k == 0 else 'V', dst=('ytx', k))
                hw0 += ch[k]

    ntail = NTAPER + 1
    nbody = len(raw) - ntail
    for i, j in enumerate(raw):
        if i < nbody:
            j['slot'] = i % nbuf
            j['boff'] = 0
            j['q'] = 'S'
            j['sem'] = j['slot']
            j['wait'] = 16 * (i // nbuf + 1)
            j['gate'] = i - nbuf if i >= nbuf else None
        else:
            t = i - nbody
            j['slot'] = 0 if t == 0 else 1
            j['boff'] = 0 if t == 0 else j['hw0']
            j['q'] = 'S'
            j['sem'] = nbuf + t
            j['wait'] = 16
            # gate on the consumer of that slot's last body occupant
            last_body = max(b for b in range(nbody) if b % nbuf == j['slot'])
            j['gate'] = last_body

    # producer (sem kind, cumulative count) per job + per-ct counts.
    # An ACT job issues one activation per batch row (accum_out is one
    # column), so it increments act_sem nb times.
    vcount = acount = 0
    ct_vdone = [0] * CT
    ct_adone = [0] * CT
    for j in raw:
        if j['eng'] == 'V':
            vcount += 1
            j['prod'] = ('V', vcount)
        else:
            acount += j['nb']
            j['prod'] = ('A', acount)
        ct_vdone[j['ct']] = vcount
        ct_adone[j['ct']] = acount
    return raw, ct_vdone, ct_adone, vcount, acount, ntail


def build_nc(hw: int = HW, nbuf: int = NBUF):
    nc = bass.Bass(enable_partition_id=False, monotonic_sem_count=0)
    BF16 = mybir.dt.bfloat16

    x_e = nc.declare_dram_parameter("x", [BL, CT, 128, hw], F32, isOutput=False)
    w1t_e = nc.declare_dram_parameter("w1t", [128, CT, HIDE], F32, isOutput=False)
    a2_e = nc.declare_dram_parameter("a2", [HIDE, HIDE], BF16, isOutput=False)
    w4t_e = nc.declare_dram_parameter("w4t", [HIDE, OP], BF16, isOutput=False)
    scal_e = nc.declare_dram_parameter("scal", [BL, 2], F32, isOutput=False)
    eye_e = nc.declare_dram_parameter("eye8", [BL, BL], F32, isOutput=False)
    out_e = nc.declare_dram_parameter("out", [BL, OP], F32, isOutput=True)

    Exp = mybir.ActivationFunctionType.Exp
    Tanh = mybir.ActivationFunctionType.Tanh
    Copy = mybir.ActivationFunctionType.Copy

    from contextlib import ExitStack

    with ExitStack() as ctx:
        bufs = [
            ctx.enter_context(nc.sbuf_tensor(f"buf{j}", [128, NBST, hw], BF16))
            for j in range(nbuf)
        ]
        yt = ctx.enter_context(nc.sbuf_tensor("yt", [128, CT, BL], F32))
        ytx = ctx.enter_context(nc.sbuf_tensor("ytx", [128, NTAPER], F32))
        waste = ctx.enter_context(
            nc.sbuf_tensor("waste", [128, 2, hw], BF16)
        )
        w1ts = ctx.enter_context(nc.sbuf_tensor("w1ts", [128, CT, HIDE], F32))
        a2s = ctx.enter_context(nc.sbuf_tensor("a2s", [HIDE, HIDE], BF16))
        w4ts = ctx.enter_context(nc.sbuf_tensor("w4ts", [HIDE, OP], BF16))
        scals = ctx.enter_context(nc.sbuf_tensor("scals", [BL, 2], F32))
        eyes = ctx.enter_context(nc.sbuf_tensor("eyes", [BL, BL], F32))
        de1 = ctx.enter_context(nc.sbuf_tensor("de1", [1, 1], F32))

        y1ts = ctx.enter_context(nc.sbuf_tensor("y1ts", [HIDE, BL], BF16))
        es = ctx.enter_context(nc.sbuf_tensor("es", [BL, HIDE], F32))
        ss = ctx.enter_context(nc.sbuf_tensor("ss", [BL, 1], F32))
        rs = ctx.enter_context(nc.sbuf_tensor("rs", [BL, 1], F32))
        t1s = ctx.enter_context(nc.sbuf_tensor("t1s", [BL, HIDE], F32))
        y2s = ctx.enter_context(nc.sbuf_tensor("y2s", [BL, HIDE], F32))
        y3s = ctx.enter_context(nc.sbuf_tensor("y3s", [BL, HIDE], F32))
        y3ts = ctx.enter_context(nc.sbuf_tensor("y3ts", [HIDE, BL], BF16))
        esig = ctx.enter_context(nc.sbuf_tensor("esig", [BL, OP], F32))
        outs = ctx.enter_context(nc.sbuf_tensor("outs", [BL, OP], F32))

        y1_ps = ctx.enter_context(nc.psum_tensor("y1_ps", [BL, HIDE], F32))
        y1t_ps = ctx.enter_context(nc.psum_tensor("y1t_ps", [HIDE, BL], F32))
        p2_ps = ctx.enter_context(nc.psum_tensor("p2_ps", [BL, HIDE], F32))
        y3t_ps = ctx.enter_context(nc.psum_tensor("y3t_ps", [HIDE, BL], F32))
        o_ps = ctx.enter_context(nc.psum_tensor("o_ps", [BL, OP], F32))

        jobs, ct_vdone, ct_adone, NV, NA, ntail = make_jobs(hw, nbuf)
        njobs = len(jobs)
        R0 = NV + 1        # red_sem once yt complete (all V reduces + combine)
        AEXP = NA + 1      # act_sem count of the epilogue exp

        dma_sems = [
            ctx.enter_context(nc.semaphore(f"dma_sem{j}"))
            for j in range(nbuf + ntail)
        ]
        out_sem = ctx.enter_context(nc.semaphore("out_sem"))
        param_sem = ctx.enter_context(nc.semaphore("param_sem"))
        red_sem = ctx.enter_context(nc.semaphore("red_sem"))
        pe_sem = ctx.enter_context(nc.semaphore("pe_sem"))
        act_sem = ctx.enter_context(nc.semaphore("act_sem"))
        sem_of = {'V': red_sem, 'A': act_sem}

        def issue_stream(eng, q):
            for i, j in enumerate(jobs):
                if j['q'] != q:
                    continue
                if j['gate'] is not None:
                    pk, pc = jobs[j['gate']]['prod']
                    eng.wait_ge(sem_of[pk], pc)
                src = x_e[
                    j['b0']:j['b0'] + j['nb'], j['ct'], :,
                    j['hw0']:j['hw0'] + j['nhw']
                ].rearrange("b p w -> p b w")
                dstap = bufs[j['slot']][
                    :, 0:j['nb'], j['boff']:j['boff'] + j['nhw']
                ]
                eng.dma_start(out=dstap, in_=src).then_inc(dma_sems[j['sem']], 16)

        def buf_in(j):
            return bufs[j['slot']][
                :, 0:j['nb'], j['boff']:j['boff'] + j['nhw']
            ]

        with nc.Block() as block:

            @block.gpsimd
            def _(gpsimd):
                # SWDGE stream: casts f32 DRAM -> bf16 SBUF in the DMA
                # datapath, halving the SBUF-AXI write bytes.
                issue_stream(gpsimd, 'S')

            @block.sync
            def _(sync):
                # Output DMA once DVE finishes the sigmoid tail.
                sync.wait_ge(red_sem, R0 + 6)
                sync.dma_start(out=out_e[:, :], in_=outs[:, :]).then_inc(out_sem, 16)
                sync.wait_ge(out_sem, 16)

            @block.scalar
            def _(scalar):
                # Param loads lead the scalar HWDGE queue.
                scalar.dma_start(out=w1ts[:, :, :], in_=w1t_e[:, :, :]).then_inc(
                    param_sem, 16
                )
                scalar.dma_start(out=a2s[:, :], in_=a2_e[:, :]).then_inc(param_sem, 16)
                scalar.dma_start(out=w4ts[:, :], in_=w4t_e[:, :]).then_inc(
                    param_sem, 16
                )
                scalar.dma_start(out=scals[:, :], in_=scal_e[:, :]).then_inc(
                    param_sem, 16
                )
                scalar.dma_start(out=eyes[:, :], in_=eye_e[:, :]).then_inc(
                    param_sem, 16
                )
                # Preload the exp/tanh table set during the stream.
                c0 = nc.const_aps.tensor(0.0, (1, 1))
                scalar.activation(de1[:, :], c0, Exp)
                # This queue also carries half the body stream.
                issue_stream(scalar, 'C')
                # Reduce assists: free-dim sums via accum_out, one call per
                # batch row. Two waste regions rotate; a self-wait orders the
                # region reuse for the pipeline.
                acalls = 0
                region_last = [0, 0]
                for j in jobs:
                    if j['eng'] != 'A':
                        continue
                    scalar.wait_ge(dma_sems[j['sem']], j['wait'])
                    for b in range(j['nb']):
                        reg = acalls % 2
                        if region_last[reg] > 0:
                            scalar.wait_ge(act_sem, region_last[reg])
                        acc = (
                            yt[:, j['ct'],
                               j['dst'][1] + b:j['dst'][1] + b + 1]
                            if j['dst'][0] == 'yt'
                            else ytx[:, j['dst'][1]:j['dst'][1] + 1]
                        )
                        scalar.activation(
                            waste[:, reg, 0:j['nhw']],
                            buf_in(j)[:, b, :],
                            Copy,
                            accum_out=acc,
                        ).then_inc(act_sem, 1)
                        acalls += 1
                        region_last[reg] = acalls
                # Epilogue: exp(w2*y1) with fused softmax denominator,
                # reading y1 straight out of PSUM.
                scalar.wait_ge(param_sem, 80)
                scalar.wait_ge(pe_sem, 8)
                scalar.activation(
                    es[:, :], y1_ps[:, :], Exp, scale=scals[:, 0:1],
                    accum_out=ss[:, :],
                ).then_inc(act_sem, 1)
                scalar.wait_ge(pe_sem, 10)
                scalar.copy(y3ts[:, :], y3t_ps[:, :]).then_inc(act_sem, 1)
                # sigmoid(z) = 0.5*tanh(z/2) + 0.5 (tanh shares the exp set).
                scalar.wait_ge(pe_sem, 11)
                scalar.activation(
                    esig[:, :], o_ps[:, :], Tanh, scale=0.5
                ).then_inc(act_sem, 1)

            @block.vector
            def _(vector):
                for j in jobs:
                    if j['eng'] != 'V':
                        continue
                    vector.wait_ge(dma_sems[j['sem']], j['wait'])
                    out_ap = (
                        yt[:, j['ct'], j['dst'][1]:j['dst'][1] + j['dst'][2]]
                        if j['dst'][0] == 'yt'
                        else ytx[:, j['dst'][1]:j['dst'][1] + 1]
                    )
                    vector.reduce_sum(
                        out_ap, buf_in(j), axis=mybir.AxisListType.X
                    ).then_inc(red_sem, 1)
                # Combine the taper partials: yt[:, CT-1, BL-1] = sum(ytx)
                vector.wait_ge(red_sem, NV)
                vector.wait_ge(act_sem, NA)
                vector.reduce_sum(
                    yt[:, CT - 1, BL - 1:BL], ytx[:, :],
                    axis=mybir.AxisListType.X,
                ).then_inc(red_sem, 1)
                # Epilogue. y1ts copy (f32->bf16) runs on DVE.
                vector.wait_ge(pe_sem, 7)
                vector.tensor_copy(y1ts[:, :], y1t_ps[:, :]).then_inc(red_sem, 1)
                vector.wait_ge(act_sem, AEXP)
                vector.reciprocal(rs[:, :], ss[:, :]).then_inc(red_sem, 1)
                vector.wait_ge(red_sem, R0 + 2)
                # t1 = (es * 1/s) * y1  (y1 read from PSUM)
                vector.scalar_tensor_tensor(
                    t1s[:, :], es[:, :], rs[:, 0:1], y1_ps[:, :],
                    op0=mybir.AluOpType.mult, op1=mybir.AluOpType.mult,
                ).then_inc(red_sem, 1)
                vector.wait_ge(pe_sem, 9)
                vector.wait_ge(red_sem, R0 + 3)
                vector.tensor_add(y2s[:, :], t1s[:, :], p2_ps[:, :]).then_inc(
                    red_sem, 1
                )
                vector.wait_ge(red_sem, R0 + 4)
                vector.tensor_scalar(
                    y3s[:, :],
                    y2s[:, :],
                    scals[:, 1:2],
                    0.0,
                    op0=mybir.AluOpType.mult,
                    op1=mybir.AluOpType.max,
                ).then_inc(red_sem, 1)
                # Final sigmoid tail: outs = 0.5*tanh + 0.5
                vector.wait_ge(act_sem, AEXP + 2)
                vector.tensor_scalar(
                    outs[:, :], esig[:, :], 0.5, 0.5,
                    op0=mybir.AluOpType.mult, op1=mybir.AluOpType.add,
                ).then_inc(red_sem, 1)

            @block.tensor
            def _(tensor):
                tensor.wait_ge(param_sem, 80)
                # W1 matmuls per channel tile, issued as soon as that tile of
                # yt is fully reduced (overlaps the remaining stream).
                for ct in range(CT):
                    if ct < CT - 1:
                        tensor.wait_ge(red_sem, ct_vdone[ct])
                        if ct_adone[ct] > 0:
                            tensor.wait_ge(act_sem, ct_adone[ct])
                    else:
                        tensor.wait_ge(red_sem, R0)
                    tensor.matmul(
                        y1t_ps[:, :],
                        w1ts[:, ct, :],
                        yt[:, ct, :],
                        start=(ct == 0),
                        stop=(ct == CT - 1),
                    ).then_inc(pe_sem, 1)
                    tensor.matmul(
                        y1_ps[:, :],
                        yt[:, ct, :],
                        w1ts[:, ct, :],
                        start=(ct == 0),
                        stop=(ct == CT - 1),
                    ).then_inc(pe_sem, 1)
                # p2[b, k] = sum_h y1T[h, b] * A2[h, k]
                tensor.wait_ge(red_sem, R0 + 1)
                tensor.matmul(
                    p2_ps[:, :], y1ts[:, :], a2s[:, :], start=True, stop=True
                ).then_inc(pe_sem, 1)
                # y3T = transpose(y3)
                tensor.wait_ge(red_sem, R0 + 5)
                tensor.transpose(y3t_ps[:, :], y3s[:, :], eyes[:, :]).then_inc(
                    pe_sem, 1
                )
                # out[b, o] = sum_h y3T[h, b] * W4T[h, o]
                tensor.wait_ge(act_sem, AEXP + 1)
                tensor.matmul(
                    o_ps[:, :], y3ts[:, :], w4ts[:, :], start=True, stop=True
                ).then_inc(pe_sem, 1)

    return nc


def prep_in_maps(x, W1, A2, w2, w3, W4, hw: int = HW):
    """Shard x over batch; replicate (pre-transposed) params."""
    x = np.ascontiguousarray(np.asarray(x, dtype=np.float32))
    # W1T with the mean scale folded in: [c, h] -> [128, CT, HIDE] with
    # w1t[p, ct, h] = W1[h, ct*128+p] / hw
    w1t = np.ascontiguousarray(
        (np.asarray(W1, np.float32).T / hw).reshape(CT, 128, HIDE).transpose(1, 0, 2)
    )
    import ml_dtypes

    a2 = np.ascontiguousarray(np.asarray(A2, np.float32)).astype(ml_dtypes.bfloat16)
    w4t = np.ascontiguousarray(np.asarray(W4, np.float32).T).astype(
        ml_dtypes.bfloat16
    )
    scal = np.empty((BL, 2), np.float32)
    scal[:, 0] = np.float32(w2)
    scal[:, 1] = np.float32(w3)
    eye8 = np.eye(BL, dtype=np.float32)

    in_maps = []
    for c in range(NCORES):
        xs = x[c * BL:(c + 1) * BL].reshape(BL, CT, 128, hw)
        in_maps.append(
            {
                "x": xs,
                "w1t": w1t,
                "a2": a2,
                "w4t": w4t,
                "scal": scal,
                "eye8": eye8,
            }
        )
    return in_maps


def run(inputs: dict, trace: bool = False, tmpdir: str | None = None,
        trace_cores=None):
    """Build + run on 8 cores. Returns (full_output, BassKernelResults)."""
    nc = build_nc()
    in_maps = prep_in_maps(
        inputs["x"], inputs["W1"], inputs["A2"], inputs["w2"], inputs["w3"],
        inputs["W4"],
    )
    res = run_bass_kernel_spmd(
        nc, in_maps, core_ids=list(range(NCORES)), trace=trace, tmpdir=tmpdir,
        trace_cores=trace_cores,
    )
    out = np.concatenate([res.results[c]["out"] for c in range(NCORES)], axis=0)
    return out.reshape(B, OP, 1, 1).astype(np.float32), res


def kernel(**inputs) -> np.ndarray:
    out, _ = run(inputs, trace=False)
    return out


# revision 46
# speedup vs baseline: 1.0012x; 1.0012x over previous
"""AGCA (adaptive graph channel attention) distributed Bass kernel for TRN2.

Reference computation (per batch row b):
    y   = mean(x[b], axis=(H,W))                    # [CIN]
    y1  = W1 @ y                                    # [HIDE]
    A1  = softmax(w2 * y1)                          # [HIDE]
    y2  = y1 * A1 + A2.T-contract(y1)               # y1@A2
    y3  = relu(w3 * y2)
    out = sigmoid(W4 @ y3)                          # [OP]

Sharding: pure data-parallel over batch. Each of the 8 cores handles
B/8 = 8 batch rows end-to-end; the tiny params are replicated. No
collectives. The kernel is memory-bound on streaming x (64 MiB/core).

Per-core dataflow:
  - x shard viewed as [BL=8, CT=4, 128, 4096] (batch, channel-tile,
    channel-within-tile, H*W), streamed ct-major as ~4 MiB supertiles
    through an NBUF-deep SBUF rotation via SWDGE DMA that casts
    f32 -> bf16 in the datapath (halves SBUF-AXI write traffic; the
    stream runs at the per-core HBM read limit, ~425 GB/s).
  - The vector engine sum-reduces each supertile along the free axis
    into yT tiles [128c, 8b] (f32 accumulation; the 1/4096 mean scale
    is folded into W1 on the host). Every 3rd body reduce and the two
    slowest tail reduces ride the otherwise-idle scalar engine
    (activation Copy with accum_out), so both engines keep >30% margin
    over the DMA cadence even when engine clocks are throttled ~20% by
    co-tenant load. The final batch row is tapered geometrically along
    hw, so the post-last-byte reduce work is under 1 us.
  - Per channel tile, its W1 matmuls run mid-stream on the tensor
    engine (y1 [8,128] and y1T [128,8] layouts both computed so softmax
    runs along the free axis). The epilogue reads y1 straight from
    PSUM: exp with fused accum (softmax denominator) on ACT, the
    normalize/A2/relu chain on DVE+PE, and sigmoid as
    0.5*tanh(z/2)+0.5 (tanh shares the exp LUT set, so no table load
    sits on the critical path).
  - Output [8, 512] (batch-major) DMAd out; host concatenates shards.
"""

import numpy as np

import concourse.bass as bass
import concourse.mybir as mybir
from concourse.bass_utils import run_bass_kernel_spmd


def _install_ntff_shim():
    """Fill in the optional antenv.axon_hooks module if the image lacks it,
    so run_bass_kernel_spmd(trace=True) (or BASS_TRACE=1) can drive NTFF
    profiling through libaxon_pjrt.so instead of crashing on the import.
    No-op when the module exists or the axon .so is unavailable."""
    import sys as _sys
    import types as _types

    if "antenv.axon_hooks" in _sys.modules:
        return
    try:
        import antenv  # noqa: F401
        import importlib.util as _ilu

        if _ilu.find_spec("antenv.axon_hooks") is not None:
            return
        mod = _types.ModuleType("antenv.axon_hooks")
        _hook = [None]
        mod.set_axon_ntff_profile_hook = lambda h: _hook.__setitem__(0, h)
        mod.get_axon_ntff_profile_hook = lambda: _hook[0]
        try:
            from trn_agent_boot.trn_boot import _ntff_profile_via_ctypes

            mod.set_axon_ntff_profile_hook(
                _ntff_profile_via_ctypes("/opt/axon/libaxon_pjrt.so")
            )
        except Exception:
            pass  # hook stays None; bass_utils logs and skips tracing
        _sys.modules["antenv.axon_hooks"] = mod
        antenv.axon_hooks = mod
    except Exception:
        pass


_install_ntff_shim()

F32 = mybir.dt.float32

B, CIN, H, W = 64, 512, 64, 64
HW = H * W          # 4096
NCORES = 8
BL = B // NCORES    # 8 batch rows per core
CT = CIN // 128     # 4 channel tiles
HIDE = 128
OP = 512
NBST = 2            # batch rows per (full) supertile
NBUF = 5            # streaming buffers


NTAPER = 5  # taper chunks for the very last batch row (1 ACT + 4 DVE)


def make_jobs(hw, nbuf):
    """Streaming schedule.

    Each job dict: b0, nb, ct, hw0, nhw, eng ('V' DVE / 'A' ACT reduce),
    dst ('yt', b0, nb) | ('ytx', k), slot (buffer index), boff (element
    offset within the slot's hw axis), sem (completion-sem index), wait (completion threshold),
    gate (job index whose consumer must finish before this DMA issues).

    Body jobs rotate through the nbuf slots. Tail jobs (b=BL-1 taper + the b=BL-2 single) use slot 0 / slot 1
    sub-regions with private sems so nothing gates on late reduces.
    """
    raw = []

    def add(**kw):
        raw.append(dict(kw))

    bi = 0  # body index; every 3rd body reduce rides ACT so the DVE
    # keeps a wide margin even when engine clocks are throttled ~20%.

    def beng():
        nonlocal bi
        e = 'A' if bi % 3 == 0 else 'V'
        bi += 1
        return e

    for ct in range(CT):
        if ct < CT - 1:
            for b0 in range(0, BL, NBST):
                add(b0=b0, nb=NBST, ct=ct, hw0=0, nhw=hw, eng=beng(),
                    dst=('yt', b0, NBST))
        else:
            for b0 in range(0, BL - NBST, NBST):
                add(b0=b0, nb=NBST, ct=ct, hw0=0, nhw=hw, eng=beng(),
                    dst=('yt', b0, NBST))
            add(b0=BL - 2, nb=1, ct=ct, hw0=0, nhw=hw, eng='A',
                dst=('yt', BL - 2, 1))
            ch = [hw // 2, hw // 4, hw // 8, hw // 16, hw // 16]
            assert len(ch) == NTAPER and sum(ch) == hw
            hw0 = 0
            for k in range(NTAPER):
                add(b0=BL - 1, nb=1, ct=ct, hw0=hw0, nhw=ch[k],
                    eng='A' if k == 0 else 'V', dst=('ytx', k))
                hw0 += ch[k]

    ntail = NTAPER + 1
    nbody = len(raw) - ntail
    for i, j in enumerate(raw):
        if i < nbody:
            j['slot'] = i % nbuf
            j['boff'] = 0
            j['sem'] = j['slot']
            j['wait'] = 16 * (i // nbuf + 1)
            j['gate'] = i - nbuf if i >= nbuf else None
        else:
            t = i - nbody
            j['slot'] = 0 if t == 0 else 1
            j['boff'] = 0 if t == 0 else j['hw0']
            j['sem'] = nbuf + t
            j['wait'] = 16
            # gate on the consumer of that slot's last body occupant
            last_body = max(b for b in range(nbody) if b % nbuf == j['slot'])
            j['gate'] = last_body

    # producer (sem kind, cumulative count) per job + per-ct counts.
    # An ACT job issues one activation per batch row (accum_out is one
    # column), so it increments act_sem nb times.
    vcount = acount = 0
    ct_vdone = [0] * CT
    ct_adone = [0] * CT
    for j in raw:
        if j['eng'] == 'V':
            vcount += 1
            j['prod'] = ('V', vcount)
        else:
            acount += j['nb']
            j['prod'] = ('A', acount)
        ct_vdone[j['ct']] = vcount
        ct_adone[j['ct']] = acount
    return raw, ct_vdone, ct_adone, vcount, acount, ntail


def build_nc(hw: int = HW, nbuf: int = NBUF):
    nc = bass.Bass(enable_partition_id=False, monotonic_sem_count=0)
    BF16 = mybir.dt.bfloat16

    x_e = nc.declare_dram_parameter("x", [BL, CT, 128, hw], F32, isOutput=False)
    w1t_e = nc.declare_dram_parameter("w1t", [128, CT, HIDE], F32, isOutput=False)
    a2_e = nc.declare_dram_parameter("a2", [HIDE, HIDE], BF16, isOutput=False)
    w4t_e = nc.declare_dram_parameter("w4t", [HIDE, OP], BF16, isOutput=False)
    scal_e = nc.declare_dram_parameter("scal", [BL, 2], F32, isOutput=False)
    eye_e = nc.declare_dram_parameter("eye8", [BL, BL], BF16, isOutput=False)
    out_e = nc.declare_dram_parameter("out", [BL, OP], F32, isOutput=True)

    Exp = mybir.ActivationFunctionType.Exp
    Tanh = mybir.ActivationFunctionType.Tanh
    Copy = mybir.ActivationFunctionType.Copy

    from contextlib import ExitStack

    with ExitStack() as ctx:
        bufs = [
            ctx.enter_context(nc.sbuf_tensor(f"buf{j}", [128, NBST, hw], BF16))
            for j in range(nbuf)
        ]
        yt = ctx.enter_context(nc.sbuf_tensor("yt", [128, CT, BL], F32))
        ytx = ctx.enter_context(nc.sbuf_tensor("ytx", [128, NTAPER], F32))
        waste = ctx.enter_context(
            nc.sbuf_tensor("waste", [128, 2, hw], BF16)
        )
        w1ts = ctx.enter_context(nc.sbuf_tensor("w1ts", [128, CT, HIDE], F32))
        a2s = ctx.enter_context(nc.sbuf_tensor("a2s", [HIDE, HIDE], BF16))
        w4ts = ctx.enter_context(nc.sbuf_tensor("w4ts", [HIDE, OP], BF16))
        scals = ctx.enter_context(nc.sbuf_tensor("scals", [BL, 2], F32))
        eyes = ctx.enter_context(nc.sbuf_tensor("eyes", [BL, BL], BF16))
        de1 = ctx.enter_context(nc.sbuf_tensor("de1", [1, 1], F32))

        y1ts = ctx.enter_context(nc.sbuf_tensor("y1ts", [HIDE, BL], BF16))
        es = ctx.enter_context(nc.sbuf_tensor("es", [BL, HIDE], F32))
        ss = ctx.enter_context(nc.sbuf_tensor("ss", [BL, 1], F32))
        rs = ctx.enter_context(nc.sbuf_tensor("rs", [BL, 1], F32))
        t1s = ctx.enter_context(nc.sbuf_tensor("t1s", [BL, HIDE], F32))
        y2s = ctx.enter_context(nc.sbuf_tensor("y2s", [BL, HIDE], BF16))
        y3s = ctx.enter_context(nc.sbuf_tensor("y3s", [BL, HIDE], F32))
        y3ts = ctx.enter_context(nc.sbuf_tensor("y3ts", [HIDE, BL], BF16))
        esig = ctx.enter_context(nc.sbuf_tensor("esig", [BL, OP], F32))
        outs = ctx.enter_context(nc.sbuf_tensor("outs", [BL, OP], F32))

        y1_ps = ctx.enter_context(nc.psum_tensor("y1_ps", [BL, HIDE], F32))
        y1t_ps = ctx.enter_context(nc.psum_tensor("y1t_ps", [HIDE, BL], F32))
        p2_ps = ctx.enter_context(nc.psum_tensor("p2_ps", [BL, HIDE], F32))
        y3t_ps = ctx.enter_context(nc.psum_tensor("y3t_ps", [HIDE, BL], F32))
        o_ps = ctx.enter_context(nc.psum_tensor("o_ps", [BL, OP], F32))

        jobs, ct_vdone, ct_adone, NV, NA, ntail = make_jobs(hw, nbuf)
        njobs = len(jobs)
        R0 = NV + 1        # red_sem once yt complete (all V reduces + combine)
        AEXP = NA + 1      # act_sem count of the epilogue exp

        dma_sems = [
            ctx.enter_context(nc.semaphore(f"dma_sem{j}"))
            for j in range(nbuf + ntail)
        ]
        out_sem = ctx.enter_context(nc.semaphore("out_sem"))
        param_sem = ctx.enter_context(nc.semaphore("param_sem"))
        red_sem = ctx.enter_context(nc.semaphore("red_sem"))
        pe_sem = ctx.enter_context(nc.semaphore("pe_sem"))
        act_sem = ctx.enter_context(nc.semaphore("act_sem"))
        sem_of = {'V': red_sem, 'A': act_sem}

        def buf_in(j):
            return bufs[j['slot']][
                :, 0:j['nb'], j['boff']:j['boff'] + j['nhw']
            ]

        def issue_stream(eng):
            for j in jobs:
                if j['gate'] is not None:
                    pk, pc = jobs[j['gate']]['prod']
                    eng.wait_ge(sem_of[pk], pc)
                src = x_e[
                    j['b0']:j['b0'] + j['nb'], j['ct'], :,
                    j['hw0']:j['hw0'] + j['nhw']
                ].rearrange("b p w -> p b w")
                eng.dma_start(out=buf_in(j), in_=src).then_inc(
                    dma_sems[j['sem']], 16
                )

        with nc.Block() as block:

            @block.gpsimd
            def _(gpsimd):
                # SWDGE stream: casts f32 DRAM -> bf16 SBUF in the DMA
                # datapath, halving the SBUF-AXI write bytes.
                issue_stream(gpsimd)

            @block.sync
            def _(sync):
                # Output DMA once both sigmoid halves land in SBUF.
                sync.wait_ge(red_sem, R0 + 5)
                sync.wait_ge(act_sem, AEXP + 4)
                sync.dma_start(out=out_e[:, :], in_=outs[:, :]).then_inc(out_sem, 16)
                sync.wait_ge(out_sem, 16)

            @block.scalar
            def _(scalar):
                # Param loads lead the scalar HWDGE queue.
                scalar.dma_start(out=w1ts[:, :, :], in_=w1t_e[:, :, :]).then_inc(
                    param_sem, 16
                )
                scalar.dma_start(out=a2s[:, :], in_=a2_e[:, :]).then_inc(param_sem, 16)
                scalar.dma_start(out=w4ts[:, :], in_=w4t_e[:, :]).then_inc(
                    param_sem, 16
                )
                scalar.dma_start(out=scals[:, :], in_=scal_e[:, :]).then_inc(
                    param_sem, 16
                )
                scalar.dma_start(out=eyes[:, :], in_=eye_e[:, :]).then_inc(
                    param_sem, 16
                )
                # Preload the exp/tanh table set during the stream.
                c0 = nc.const_aps.tensor(0.0, (1, 1))
                scalar.activation(de1[:, :], c0, Exp)
                # Reduce assists: free-dim sums via accum_out, one call per
                # batch row. Two waste regions rotate; a self-wait orders the
                # region reuse for the pipeline.
                acalls = 0
                region_last = [0, 0]
                for j in jobs:
                    if j['eng'] != 'A':
                        continue
                    scalar.wait_ge(dma_sems[j['sem']], j['wait'])
                    for b in range(j['nb']):
                        reg = acalls % 2
                        if region_last[reg] > 0:
                            scalar.wait_ge(act_sem, region_last[reg])
                        acc = (
                            yt[:, j['ct'],
                               j['dst'][1] + b:j['dst'][1] + b + 1]
                            if j['dst'][0] == 'yt'
                            else ytx[:, j['dst'][1]:j['dst'][1] + 1]
                        )
                        scalar.activation(
                            waste[:, reg, 0:j['nhw']],
                            buf_in(j)[:, b, :],
                            Copy,
                            accum_out=acc,
                        ).then_inc(act_sem, 1)
                        acalls += 1
                        region_last[reg] = acalls
                # Epilogue: exp(w2*y1) with fused softmax denominator,
                # reading y1 straight out of PSUM.
                scalar.wait_ge(param_sem, 80)
                scalar.wait_ge(pe_sem, 7)
                scalar.activation(
                    es[:, :], y1_ps[:, :], Exp, scale=scals[:, 0:1],
                    accum_out=ss[:, :],
                ).then_inc(act_sem, 1)
                scalar.wait_ge(pe_sem, 10)
                scalar.activation(
                    y3ts[:, :], y3t_ps[:, :],
                    mybir.ActivationFunctionType.Relu,
                ).then_inc(act_sem, 1)
                # sigmoid(z) = 0.5*tanh(z/2) + 0.5 (tanh shares the exp
                # set). Column-half pipeline: ACT tanh h1, then tanh h2
                # while DVE applies h1's scale/bias; ACT finishes h2.
                scalar.wait_ge(pe_sem, 11)
                scalar.activation(
                    esig[:, 0:OP // 2], o_ps[:, 0:OP // 2], Tanh, scale=0.5
                ).then_inc(act_sem, 1)
                scalar.wait_ge(pe_sem, 12)
                scalar.activation(
                    esig[:, OP // 2:OP], o_ps[:, OP // 2:OP], Tanh, scale=0.5
                ).then_inc(act_sem, 1)
                scalar.wait_ge(act_sem, AEXP + 3)
                scalar.activation(
                    outs[:, OP // 2:OP], esig[:, OP // 2:OP], Copy,
                    scale=0.5, bias=0.5,
                ).then_inc(act_sem, 1)

            @block.vector
            def _(vector):
                for j in jobs:
                    if j['eng'] != 'V':
                        continue
                    vector.wait_ge(dma_sems[j['sem']], j['wait'])
                    out_ap = (
                        yt[:, j['ct'], j['dst'][1]:j['dst'][1] + j['dst'][2]]
                        if j['dst'][0] == 'yt'
                        else ytx[:, j['dst'][1]:j['dst'][1] + 1]
                    )
                    vector.reduce_sum(
                        out_ap, buf_in(j), axis=mybir.AxisListType.X
                    ).then_inc(red_sem, 1)
                # Combine the taper partials: yt[:, CT-1, BL-1] = sum(ytx)
                vector.wait_ge(red_sem, NV)
                vector.wait_ge(act_sem, NA)
                vector.reduce_sum(
                    yt[:, CT - 1, BL - 1:BL], ytx[:, :],
                    axis=mybir.AxisListType.X,
                ).then_inc(red_sem, 1)
                # Epilogue. y1ts copy (f32->bf16) runs on DVE.
                vector.wait_ge(pe_sem, 8)
                vector.tensor_copy(y1ts[:, :], y1t_ps[:, :]).then_inc(red_sem, 1)
                vector.wait_ge(act_sem, AEXP)
                vector.reciprocal(rs[:, :], ss[:, :]).then_inc(red_sem, 1)
                vector.wait_ge(red_sem, R0 + 2)
                # t1 = (es * 1/s) * y1  (y1 read from PSUM)
                vector.scalar_tensor_tensor(
                    t1s[:, :], es[:, :], rs[:, 0:1], y1_ps[:, :],
                    op0=mybir.AluOpType.mult, op1=mybir.AluOpType.mult,
                ).then_inc(red_sem, 1)
                vector.wait_ge(pe_sem, 9)
                vector.wait_ge(red_sem, R0 + 3)
                vector.tensor_add(y2s[:, :], t1s[:, :], p2_ps[:, :]).then_inc(
                    red_sem, 1
                )
                # Sigmoid tail, first half: outs_h1 = 0.5*tanh_h1 + 0.5
                vector.wait_ge(act_sem, AEXP + 2)
                vector.tensor_scalar(
                    outs[:, 0:OP // 2], esig[:, 0:OP // 2], 0.5, 0.5,
                    op0=mybir.AluOpType.mult, op1=mybir.AluOpType.add,
                ).then_inc(red_sem, 1)

            @block.tensor
            def _(tensor):
                tensor.wait_ge(param_sem, 80)
                # W1 matmuls per channel tile, issued as soon as that tile of
                # yt is fully reduced (overlaps the remaining stream).
                for ct in range(CT):
                    if ct < CT - 1:
                        tensor.wait_ge(red_sem, ct_vdone[ct])
                        if ct_adone[ct] > 0:
                            tensor.wait_ge(act_sem, ct_adone[ct])
                    else:
                        tensor.wait_ge(red_sem, R0)
                    tensor.matmul(
                        y1_ps[:, :],
                        yt[:, ct, :],
                        w1ts[:, ct, :],
                        start=(ct == 0),
                        stop=(ct == CT - 1),
                    ).then_inc(pe_sem, 1)
                    tensor.matmul(
                        y1t_ps[:, :],
                        w1ts[:, ct, :],
                        yt[:, ct, :],
                        start=(ct == 0),
                        stop=(ct == CT - 1),
                    ).then_inc(pe_sem, 1)
                # p2[b, k] = sum_h y1T[h, b] * A2[h, k]
                tensor.wait_ge(red_sem, R0 + 1)
                tensor.matmul(
                    p2_ps[:, :], y1ts[:, :], a2s[:, :], start=True, stop=True
                ).then_inc(pe_sem, 1)
                # w3*y2T via matmul with the w3-scaled identity
                tensor.wait_ge(red_sem, R0 + 4)
                tensor.matmul(
                    y3t_ps[:, :], y2s[:, :], eyes[:, :], start=True, stop=True
                ).then_inc(pe_sem, 1)
                # out[b, o] = sum_h y3T[h, b] * W4T[h, o], in column halves
                # so the sigmoid tail pipelines across ACT and DVE.
                tensor.wait_ge(act_sem, AEXP + 1)
                tensor.matmul(
                    o_ps[:, 0:OP // 2], y3ts[:, :], w4ts[:, 0:OP // 2],
                    start=True, stop=True, skip_group_check=True,
                ).then_inc(pe_sem, 1)
                tensor.matmul(
                    o_ps[:, OP // 2:OP], y3ts[:, :], w4ts[:, OP // 2:OP],
                    start=True, stop=True, skip_group_check=True,
                ).then_inc(pe_sem, 1)

    return nc


def prep_in_maps(x, W1, A2, w2, w3, W4, hw: int = HW):
    """Shard x over batch; replicate (pre-transposed) params."""
    x = np.ascontiguousarray(np.asarray(x, dtype=np.float32))
    # W1T with the mean scale folded in: [c, h] -> [128, CT, HIDE] with
    # w1t[p, ct, h] = W1[h, ct*128+p] / hw
    w1t = np.ascontiguousarray(
        (np.asarray(W1, np.float32).T / hw).reshape(CT, 128, HIDE).transpose(1, 0, 2)
    )
    import ml_dtypes

    a2 = np.ascontiguousarray(np.asarray(A2, np.float32)).astype(ml_dtypes.bfloat16)
    w4t = np.ascontiguousarray(np.asarray(W4, np.float32).T).astype(
        ml_dtypes.bfloat16
    )
    scal = np.empty((BL, 2), np.float32)
    scal[:, 0] = np.float32(w2)
    scal[:, 1] = np.float32(w3)
    # w3 folded into the transpose identity: the PE transpose-matmul then
    # produces w3*y2^T and the ACT copy applies relu.
    eye8 = (np.eye(BL) * np.float32(w3)).astype(ml_dtypes.bfloat16)

    in_maps = []
    for c in range(NCORES):
        xs = x[c * BL:(c + 1) * BL].reshape(BL, CT, 128, hw)
        in_maps.append(
            {
                "x": xs,
                "w1t": w1t,
                "a2": a2,
                "w4t": w4t,
                "scal": scal,
                "eye8": eye8,
            }
        )
    return in_maps


def run(inputs: dict, trace: bool = False, tmpdir: str | None = None,
        trace_cores=None):
    """Build + run on 8 cores. Returns (full_output, BassKernelResults)."""
    nc = build_nc()
    in_maps = prep_in_maps(
        inputs["x"], inputs["W1"], inputs["A2"], inputs["w2"], inputs["w3"],
        inputs["W4"],
    )
    res = run_bass_kernel_spmd(
        nc, in_maps, core_ids=list(range(NCORES)), trace=trace, tmpdir=tmpdir,
        trace_cores=trace_cores,
    )
    out = np.concatenate([res.results[c]["out"] for c in range(NCORES)], axis=0)
    return out.reshape(B, OP, 1, 1).astype(np.float32), res


def kernel(**inputs) -> np.ndarray:
    out, _ = run(inputs, trace=False)
    return out
